# revision 1
# baseline (speedup 1.0000x reference)
"""DglGraphConvolution Trainium2 kernel — pure-matmul aggregation.

Per graph:
  1. PE: hidden = text @ W; kept in SBUF as bf16 [128, 32, 130]
     (32 windows of 128 node rows; col 128 = 1.0 degree lane, col 129 = 0).
  2. Edges sorted by (dst_window, src_window) into 32x32 blocks of the
     128x128 node grid; each block padded to exactly T_B=2 tiles of 128
     edge lanes (pad lanes have all-zero one-hot columns).
  3. For tile t (block b = t//2, ws = b % 32, wd = b // 32):
       mm1: gath_ps[128 lane, 130] = srcOH_t^T @ hidsb[:, ws, :]
            (lhsT = srcOH [128 src-node, 128 lane] bf16, shipped)
       copy: gath_sb bf16 <- gath_ps  (DVE/ACT)
       mm2: agg_ps[128 node, 130] += dstOH_t^T @ gath_sb
            (lhsT = dstOH [128 lane, 128 dst-node] bf16, shipped)
     agg_ps accumulates over the 64 tiles of each dst window; col 128 = deg.
  4. DVE: out = agg * 1/(deg+1) + bias per window.

Everything on device uses only plain DMA loads, matmuls, and elementwise
ops. Host-side work: sharding plus a bijective re-encoding of the edge
index lists into one-hot tiles (no arithmetic touches model data).
"""

import numpy as np

B, N, E, F = 16, 4096, 131072, 128
NCORES = 8
GPC = B // NCORES  # graphs per core
W = 128  # window size
NW = N // W  # 32
T_B = 2  # tiles per (wd, ws) block; Poisson(128) <= 256 w.p. ~1
T = NW * NW * T_B  # 2048 tiles per graph
HC = 130  # feature row: 128 | ones | pad
TPC = 64  # tiles per OH load chunk (= one dst window)
NCHUNK = T // TPC  # 32

_cache = {}


def _build_program():
    from contextlib import ExitStack

    import concourse.bacc as bacc
    import concourse.tile as tile
    from concourse import mybir
    from concourse._compat import get_trn_type
    from concourse.masks import make_identity

    f32 = mybir.dt.float32
    bf16 = mybir.dt.bfloat16

    nc = bacc.Bacc(get_trn_type() or "TRN2", target_bir_lowering=False, debug=False)

    text_d = nc.dram_tensor("text", [GPC, N, F], f32, kind="ExternalInput")
    w_d = nc.dram_tensor("weight", [F, F], f32, kind="ExternalInput")
    bias_d = nc.dram_tensor("biasrep", [128, F], f32, kind="ExternalInput")
    # pre-swizzled one-hots: [graph, chunk, lane/node, tile-in-chunk, 128]
    soh_d = nc.dram_tensor(
        "srcoh", [GPC, NCHUNK, 128, TPC, 128], bf16, kind="ExternalInput"
    )
    doh_d = nc.dram_tensor(
        "dstoh", [GPC, NCHUNK, 128, TPC, 128], bf16, kind="ExternalInput"
    )
    out_d = nc.dram_tensor("out", [GPC, N, F], f32, kind="ExternalOutput")

    with tile.TileContext(nc) as tc, ExitStack() as ctx:
        const = ctx.enter_context(tc.tile_pool(name="const", bufs=1))
        work = ctx.enter_context(tc.tile_pool(name="work", bufs=3))
        hpool = ctx.enter_context(tc.tile_pool(name="hpool", bufs=1))
        ohpool = ctx.enter_context(tc.tile_pool(name="ohp", bufs=2))
        gsb = ctx.enter_context(tc.tile_pool(name="gsb", bufs=4))
        psum = ctx.enter_context(tc.tile_pool(name="psum", bufs=1, space="PSUM"))
        gpsum = ctx.enter_context(tc.tile_pool(name="gpsum", bufs=3, space="PSUM"))
        apsum = ctx.enter_context(tc.tile_pool(name="apsum", bufs=1, space="PSUM"))

        ident = const.tile([128, 128], f32)
        make_identity(nc, ident[:])
        w_sb = const.tile([128, F], f32)
        nc.sync.dma_start(w_sb[:], w_d[:, :])
        bias_sb = const.tile([128, F], f32)
        nc.sync.dma_start(bias_sb[:], bias_d[:, :])

        agg_tiles = {}
        for g in range(GPC):
            # hidden = text @ W -> SBUF bf16 [128, 32, 130]
            hidsb = hpool.tile([128, NW, HC], bf16, tag="hidsb")
            nc.vector.memset(hidsb[:], 1.0)
            for c in range(NW):
                ttile = work.tile([128, F], f32, tag="text")
                nc.sync.dma_start(ttile[:], text_d[g, 128 * c : 128 * (c + 1), :])
                tT_ps = psum.tile([128, 128], f32, tag="tT")
                nc.tensor.transpose(out=tT_ps[:], in_=ttile[:], identity=ident[:])
                tT_sb = work.tile([128, 128], f32, tag="tTs")
                nc.vector.tensor_copy(tT_sb[:], tT_ps[:])
                h_ps = psum.tile([128, F], f32, tag="h")
                nc.tensor.matmul(
                    out=h_ps[:], lhsT=tT_sb[:], rhs=w_sb[:], start=True, stop=True
                )
                nc.scalar.activation(
                    hidsb[:, c, 0:F], h_ps[:], mybir.ActivationFunctionType.Copy
                )
                nc.vector.memset(hidsb[:, c, F + 1 : HC], 0.0)

            for chunk in range(NCHUNK):
                soh = ohpool.tile([128, TPC, 128], bf16, tag="soh")
                nc.sync.dma_start(soh[:], soh_d[g, chunk])
                doh = ohpool.tile([128, TPC, 128], bf16, tag="doh")
                nc.sync.dma_start(doh[:], doh_d[g, chunk])
                for tt in range(TPC):
                    t = chunk * TPC + tt
                    blk = t // T_B
                    ws = blk % NW
                    wd = blk // NW
                    j = t % TPC  # position within the dst window (== tt)
                    gath_ps = gpsum.tile([128, HC], f32, tag="gps")
                    nc.tensor.matmul(
                        out=gath_ps[:],
                        lhsT=soh[:, tt, :],
                        rhs=hidsb[:, ws, :],
                        start=True,
                        stop=True,
                    )
                    gath_sb = gsb.tile([128, HC], bf16, tag="gsb")
                    nc.vector.tensor_copy(gath_sb[:], gath_ps[:])
                    if j == 0:
                        agg_new = apsum.tile([128, HC], f32, tag=f"agg{wd % 2}")
                        agg_tiles[wd % 2] = agg_new
                    agg_ps = agg_tiles[wd % 2]
                    nc.tensor.matmul(
                        out=agg_ps[:],
                        lhsT=doh[:, tt, :],
                        rhs=gath_sb[:],
                        start=(j == 0),
                        stop=(j == TPC - 1),
                    )
                    if j == TPC - 1:
                        rec = work.tile([128, 1], f32, tag="rec")
                        nc.vector.tensor_scalar_add(
                            rec[:], agg_ps[:, F : F + 1], 1.0
                        )
                        nc.vector.reciprocal(rec[:], rec[:])
                        o1 = work.tile([128, F], f32, tag="o1")
                        nc.vector.tensor_tensor(
                            out=o1[:],
                            in0=agg_ps[:, 0:F],
                            in1=rec[:].to_broadcast([128, F]),
                            op=mybir.AluOpType.mult,
                        )
                        o2 = work.tile([128, F], f32, tag="o2")
                        nc.vector.tensor_add(o2[:], o1[:], bias_sb[:])
                        nc.sync.dma_start(
                            out_d[g, W * wd : W * (wd + 1), :], o2[:]
                        )

    nc.compile()
    return nc


def _prep_graph(src, dst):
    """(dst_window, src_window) block sort; returns one-hot packs
    soh, doh [NCHUNK, 128, TPC, 128] float32 (cast to bf16 by caller)."""
    ws = src // W
    wd = dst // W
    blk = wd * NW + ws
    order = np.argsort(blk, kind="stable")
    s, d, bo = src[order], dst[order], blk[order]
    counts = np.bincount(bo, minlength=NW * NW)
    assert counts.max() <= T_B * 128, f"block overflow: {counts.max()}"
    soh = np.zeros((T, 128, 128), dtype=np.float32)  # [tile, node, lane]
    doh = np.zeros((T, 128, 128), dtype=np.float32)  # [tile, lane, node]
    slo = (s % W).astype(np.int64)
    dlo = (d % W).astype(np.int64)
    starts = np.zeros(NW * NW + 1, dtype=np.int64)
    np.cumsum(counts, out=starts[1:])
    pos_in_blk = np.arange(len(s)) - starts[bo]
    tile_idx = bo * T_B + pos_in_blk // 128
    lane = pos_in_blk % 128
    soh[tile_idx, slo, lane] = 1.0
    doh[tile_idx, lane, dlo] = 1.0
    soh = soh.reshape(NCHUNK, TPC, 128, 128).transpose(0, 2, 1, 3).copy()
    doh = doh.reshape(NCHUNK, TPC, 128, 128).transpose(0, 2, 1, 3).copy()
    return soh, doh


def kernel(text, weight, bias, edge_src, edge_dst):
    import ml_dtypes

    text = np.asarray(text, dtype=np.float32)
    weight = np.asarray(weight, dtype=np.float32)
    bias = np.asarray(bias, dtype=np.float32)
    edge_src = np.asarray(edge_src, dtype=np.int32)
    edge_dst = np.asarray(edge_dst, dtype=np.int32)

    if "nc" not in _cache:
        _cache["nc"] = _build_program()
    nc = _cache["nc"]

    bias_rep = np.tile(bias[None, :], (128, 1)).astype(np.float32)

    in_maps = []
    for k in range(NCORES):
        soh = np.empty((GPC, NCHUNK, 128, TPC, 128), dtype=ml_dtypes.bfloat16)
        doh = np.empty((GPC, NCHUNK, 128, TPC, 128), dtype=ml_dtypes.bfloat16)
        for g in range(GPC):
            b = k * GPC + g
            so, do = _prep_graph(edge_src[b], edge_dst[b])
            soh[g] = so.astype(ml_dtypes.bfloat16)
            doh[g] = do.astype(ml_dtypes.bfloat16)
        in_maps.append(
            {
                "text": text[k * GPC : (k + 1) * GPC],
                "weight": weight,
                "biasrep": bias_rep,
                "srcoh": soh,
                "dstoh": doh,
            }
        )

    _cache["in_maps"] = in_maps

    from concourse.bass_utils import run_bass_kernel_spmd

    res = run_bass_kernel_spmd(nc, in_maps, list(range(NCORES)))
    out = np.concatenate([res.results[k]["out"] for k in range(NCORES)], axis=0)
    return out.astype(np.float32)



# revision 6
# speedup vs baseline: 4.9753x; 4.9753x over previous
"""DglGraphConvolution Trainium2 kernel — dense-adjacency matmul.

Math:  out[b] = (A_bᵀ @ (text_b @ W)) * dinv_b + bias,  dinv = 1/(deg+1)
Reassociated as  outᵀ = Wᵀ @ (textᵀ @ A)  so every matmul uses natural
layouts (no on-chip transposes):

  1. Host ships, per graph, the dense adjacency counts A[src, dst] as
     fp8_e4m3 (exact small ints; 16.8 MB vs 128 MB of one-hot tiles),
     text as bf16, plus 1/(deg+1) replicated across partitions.
  2. PE: P[fin, dst] = Σ_ws text[ws]ᵀ @ A[ws]  — lhsT = text slab
     (natural [node, fin] layout, bf16, stationary per ws), rhs = fp8
     adjacency slab, free dim 512; all 8 PSUM banks accumulate one
     graph's full [128, 4096] P.
  3. Per 512-chunk: ACT evacuates P to bf16, one matmul with stationary
     Wᵀ... (lhsT = W natural [fin, fout]) gives outᵀ[fout, dst]; DVE
     multiplies by dinv, ACT adds per-partition bias; DMA out [F, N].
  4. Host transposes each graph's [F, N] result back to [N, F].

Data-parallel over B: 2 graphs per core, 8 cores.
"""

import numpy as np

B, N, E, F = 16, 4096, 131072, 128
NCORES = 8
GPC = B // NCORES  # graphs per core
W = 128  # src window (partition) size
NW = N // W  # 32
NB = N // 512  # 8 psum banks / 512-wide output chunks

_cache = {}


def _build_program():
    from contextlib import ExitStack

    import concourse.bacc as bacc
    import concourse.tile as tile
    from concourse import mybir
    from concourse._compat import get_trn_type

    f32 = mybir.dt.float32
    bf16 = mybir.dt.bfloat16
    f8 = mybir.dt.float8e4

    nc = bacc.Bacc(get_trn_type() or "TRN2", target_bir_lowering=False, debug=False)

    text_d = nc.dram_tensor("textb", [GPC, NW, W, F], bf16, kind="ExternalInput")
    w_d = nc.dram_tensor("weightb", [F, F], bf16, kind="ExternalInput")
    bias_d = nc.dram_tensor("biascol", [F, 1], f32, kind="ExternalInput")
    dinv_d = nc.dram_tensor("dinvrep", [GPC, 128, N], f32, kind="ExternalInput")
    adj_d = nc.dram_tensor("adj", [GPC, NW, W, N], f8, kind="ExternalInput")
    out_d = nc.dram_tensor("outT", [GPC, F, N], f32, kind="ExternalOutput")

    with tile.TileContext(nc) as tc, ExitStack() as ctx:
        const = ctx.enter_context(tc.tile_pool(name="const", bufs=1))
        tpool = ctx.enter_context(tc.tile_pool(name="tpool", bufs=2))
        dpool = ctx.enter_context(tc.tile_pool(name="dpool", bufs=2))
        mpool = ctx.enter_context(tc.tile_pool(name="mpool", bufs=4))
        ppool = ctx.enter_context(tc.tile_pool(name="ppool", bufs=3))
        opool = ctx.enter_context(tc.tile_pool(name="opool", bufs=6))
        psum = ctx.enter_context(tc.tile_pool(name="psum", bufs=1, space="PSUM"))

        w_sb = const.tile([128, F], bf16)
        nc.sync.dma_start(w_sb[:], w_d[:, :])
        bias_sb = const.tile([128, 1], f32)
        nc.sync.dma_start(bias_sb[:], bias_d[:, :])

        for g in range(GPC):
            text_sb = tpool.tile([128, NW, F], bf16, tag="text")
            for ws in range(NW):
                nc.sync.dma_start(text_sb[:, ws, :], text_d[g, ws])
            dinv_sb = dpool.tile([128, N], f32, tag="dinv")
            nc.sync.dma_start(dinv_sb[:], dinv_d[g])

            # P[fin, dst] accumulated across all ws into 8 psum banks
            P = [
                psum.tile([128, 512], f32, tag=f"P{b}", name=f"P{g}_{b}")
                for b in range(NB)
            ]
            for ws in range(NW):
                m_sb = mpool.tile([128, N], f8, tag="m")
                nc.sync.dma_start(m_sb[:], adj_d[g, ws])
                for b in range(NB):
                    nc.tensor.matmul(
                        out=P[b][:],
                        lhsT=text_sb[:, ws, :],
                        rhs=m_sb[:, 512 * b : 512 * (b + 1)],
                        start=(ws == 0),
                        stop=(ws == NW - 1),
                    )

            for b in range(NB):
                p_sb = ppool.tile([128, 512], bf16, tag="p")
                nc.scalar.activation(
                    p_sb[:], P[b][:], mybir.ActivationFunctionType.Copy
                )
                o2 = psum.tile([128, 512], f32, tag=f"P{b}", name=f"o2_{g}_{b}")
                nc.tensor.matmul(
                    out=o2[:], lhsT=w_sb[:], rhs=p_sb[:], start=True, stop=True
                )
                o1 = opool.tile([128, 512], f32, tag="o1")
                nc.vector.tensor_tensor(
                    out=o1[:],
                    in0=o2[:],
                    in1=dinv_sb[:, 512 * b : 512 * (b + 1)],
                    op=mybir.AluOpType.mult,
                )
                o3 = opool.tile([128, 512], f32, tag="o3")
                nc.scalar.activation(
                    o3[:],
                    o1[:],
                    mybir.ActivationFunctionType.Identity,
                    bias=bias_sb[:],
                )
                nc.sync.dma_start(out_d[g, :, 512 * b : 512 * (b + 1)], o3[:])

    nc.compile()
    return nc


def kernel(text, weight, bias, edge_src, edge_dst):
    import ml_dtypes

    text = np.asarray(text, dtype=np.float32)
    weight = np.asarray(weight, dtype=np.float32)
    bias = np.asarray(bias, dtype=np.float32)
    edge_src = np.asarray(edge_src, dtype=np.int64)
    edge_dst = np.asarray(edge_dst, dtype=np.int64)

    if "nc" not in _cache:
        _cache["nc"] = _build_program()
    nc = _cache["nc"]

    fp8 = ml_dtypes.float8_e4m3
    lut = np.arange(64, dtype=np.float32).astype(fp8)  # exact ints through 16+

    text_bf = text.astype(ml_dtypes.bfloat16).reshape(B, NW, W, F)
    w_bf = weight.astype(ml_dtypes.bfloat16)
    bias_col = bias.astype(np.float32).reshape(F, 1)

    in_maps = []
    for k in range(NCORES):
        adj = np.empty((GPC, NW, W, N), dtype=fp8)
        dinv = np.empty((GPC, 128, N), dtype=np.float32)
        for g in range(GPC):
            b = k * GPC + g
            cnt = np.bincount(
                edge_src[b] * N + edge_dst[b], minlength=N * N
            )
            assert cnt.max() < 16, f"edge multiplicity {cnt.max()} too large"
            adj[g] = lut[cnt].reshape(NW, W, N)
            deg = np.bincount(edge_dst[b], minlength=N).astype(np.float32)
            dinv[g] = 1.0 / (deg + 1.0)
        in_maps.append(
            {
                "textb": text_bf[k * GPC : (k + 1) * GPC],
                "weightb": w_bf,
                "biascol": bias_col,
                "dinvrep": dinv,
                "adj": adj,
            }
        )

    _cache["in_maps"] = in_maps

    from concourse.bass_utils import run_bass_kernel_spmd

    res = run_bass_kernel_spmd(nc, in_maps, list(range(NCORES)))
    out = np.empty((B, N, F), dtype=np.float32)
    for k in range(NCORES):
        for g in range(GPC):
            out[k * GPC + g] = res.results[k]["outT"][g].T
    return out


# revision 9
# speedup vs baseline: 6.5869x; 1.3239x over previous
"""DglGraphConvolution Trainium2 kernel — dense-adjacency matmul.

Math:  out[b] = (A_bᵀ @ (text_b @ W)) * dinv_b + bias,  dinv = 1/(deg+1)
Reassociated as  outᵀ = Wᵀ @ (textᵀ @ A)  so every matmul uses natural
layouts (no on-chip transposes):

  1. Host ships, per graph, the dense adjacency counts A[src, dst] as
     fp8_e4m3 (exact small ints; 16.8 MB vs 128 MB of one-hot tiles),
     text as bf16 pre-permuted to partition-major, and 1/(deg+1)
     replicated across partitions (bf16).
  2. PE: P[fin, dst] = Σ_ws text[ws]ᵀ @ A[ws]  — lhsT = text slab
     (natural [node, fin] layout, bf16, stationary per ws), rhs = fp8
     adjacency slab, free dim 512; all 8 PSUM banks accumulate one
     graph's full [128, 4096] P.
  3. Per 512-chunk: ACT evacuates P to bf16, one matmul with stationary
     W (lhsT = W natural [fin, fout]) gives outᵀ[fout, dst]; DVE
     multiplies by dinv, ACT adds per-partition bias, emits bf16;
     DMA out [F, N].
  4. Host transposes each graph's [F, N] result back to [N, F].

DMA traffic is split across both HWDGE queues (SP + Activation).
Data-parallel over B: 2 graphs per core, 8 cores.
"""

import numpy as np

B, N, E, F = 16, 4096, 131072, 128
NCORES = 8
GPC = B // NCORES  # graphs per core
W = 128  # src window (partition) size
NW = N // W  # 32
NB = N // 512  # 8 psum banks / 512-wide output chunks

_cache = {}


def _build_program():
    from contextlib import ExitStack

    import concourse.bacc as bacc
    import concourse.tile as tile
    from concourse import mybir
    from concourse._compat import get_trn_type

    f32 = mybir.dt.float32
    bf16 = mybir.dt.bfloat16
    f8 = mybir.dt.float8e4

    nc = bacc.Bacc(get_trn_type() or "TRN2", target_bir_lowering=False, debug=False)

    text_d = nc.dram_tensor("textp", [GPC, W, NW * F], bf16, kind="ExternalInput")
    w_d = nc.dram_tensor("weightb", [F, F], bf16, kind="ExternalInput")
    bias_d = nc.dram_tensor("biascol", [F, 1], f32, kind="ExternalInput")
    dinv_d = nc.dram_tensor("dinvrep", [GPC, 128, N], bf16, kind="ExternalInput")
    adj_d = nc.dram_tensor("adj", [GPC, NW, W, N], f8, kind="ExternalInput")
    out_d = nc.dram_tensor("outT", [GPC, F, N], bf16, kind="ExternalOutput")

    with tile.TileContext(nc) as tc, ExitStack() as ctx:
        const = ctx.enter_context(tc.tile_pool(name="const", bufs=1))
        tpool = ctx.enter_context(tc.tile_pool(name="tpool", bufs=2))
        dpool = ctx.enter_context(tc.tile_pool(name="dpool", bufs=2))
        mpool = ctx.enter_context(tc.tile_pool(name="mpool", bufs=6))
        ppool = ctx.enter_context(tc.tile_pool(name="ppool", bufs=3))
        opool = ctx.enter_context(tc.tile_pool(name="opool", bufs=6))
        psum = ctx.enter_context(tc.tile_pool(name="psum", bufs=1, space="PSUM"))

        w_sb = const.tile([128, F], bf16)
        nc.sync.dma_start(w_sb[:], w_d[:, :])
        bias_sb = const.tile([128, 1], f32)
        nc.sync.dma_start(bias_sb[:], bias_d[:, :])

        # prefetch both graphs' text up front (1 contiguous DMA each)
        text_sb = []
        for g in range(GPC):
            t_sb = tpool.tile([128, NW * F], bf16, tag="text", name=f"text{g}")
            nc.scalar.dma_start(t_sb[:], text_d[g])
            text_sb.append(t_sb)

        for g in range(GPC):
            dinv_sb = dpool.tile([128, N], bf16, tag="dinv", name=f"dinv{g}")
            nc.scalar.dma_start(dinv_sb[:], dinv_d[g])

            # P[fin, dst] accumulated across all ws into 8 psum banks
            P = [
                psum.tile([128, 512], f32, tag=f"P{b}", name=f"P{g}_{b}")
                for b in range(NB)
            ]
            for ws in range(NW):
                m_sb = mpool.tile([128, N], f8, tag="m", name=f"m{g}_{ws}")
                eng = nc.sync if ws % 2 == 0 else nc.scalar
                eng.dma_start(m_sb[:], adj_d[g, ws])
                for b in range(NB):
                    nc.tensor.matmul(
                        out=P[b][:],
                        lhsT=text_sb[g][:, ws * F : (ws + 1) * F],
                        rhs=m_sb[:, 512 * b : 512 * (b + 1)],
                        start=(ws == 0),
                        stop=(ws == NW - 1),
                    )

            for b in range(NB):
                p_sb = ppool.tile([128, 512], bf16, tag="p", name=f"p{g}_{b}")
                nc.scalar.activation(
                    p_sb[:], P[b][:], mybir.ActivationFunctionType.Copy
                )
                o2 = psum.tile([128, 512], f32, tag=f"P{b}", name=f"o2_{g}_{b}")
                nc.tensor.matmul(
                    out=o2[:], lhsT=w_sb[:], rhs=p_sb[:], start=True, stop=True
                )
                o1 = opool.tile([128, 512], f32, tag="o1", name=f"o1_{g}_{b}")
                nc.vector.tensor_tensor(
                    out=o1[:],
                    in0=o2[:],
                    in1=dinv_sb[:, 512 * b : 512 * (b + 1)],
                    op=mybir.AluOpType.mult,
                )
                o3 = opool.tile([128, 512], bf16, tag="o3", name=f"o3_{g}_{b}")
                nc.scalar.activation(
                    o3[:],
                    o1[:],
                    mybir.ActivationFunctionType.Identity,
                    bias=bias_sb[:],
                )
                nc.sync.dma_start(out_d[g, :, 512 * b : 512 * (b + 1)], o3[:])

    nc.compile()
    return nc


def kernel(text, weight, bias, edge_src, edge_dst):
    import ml_dtypes

    text = np.asarray(text, dtype=np.float32)
    weight = np.asarray(weight, dtype=np.float32)
    bias = np.asarray(bias, dtype=np.float32)
    edge_src = np.asarray(edge_src, dtype=np.int64)
    edge_dst = np.asarray(edge_dst, dtype=np.int64)

    if "nc" not in _cache:
        _cache["nc"] = _build_program()
    nc = _cache["nc"]

    fp8 = ml_dtypes.float8_e4m3
    bf = ml_dtypes.bfloat16
    lut = np.arange(64, dtype=np.float32).astype(fp8)  # exact ints through 16+

    # [B, N, F] -> [B, NW, 128, F] -> partition-major [B, 128, NW, F]
    text_p = np.ascontiguousarray(
        text.astype(bf).reshape(B, NW, W, F).transpose(0, 2, 1, 3)
    ).reshape(B, W, NW * F)
    w_bf = weight.astype(bf)
    bias_col = bias.astype(np.float32).reshape(F, 1)

    in_maps = []
    for k in range(NCORES):
        adj = np.empty((GPC, NW, W, N), dtype=fp8)
        dinv = np.empty((GPC, 128, N), dtype=bf)
        for g in range(GPC):
            b = k * GPC + g
            cnt = np.bincount(
                edge_src[b] * N + edge_dst[b], minlength=N * N
            )
            assert cnt.max() < 16, f"edge multiplicity {cnt.max()} too large"
            adj[g] = lut[cnt].reshape(NW, W, N)
            deg = np.bincount(edge_dst[b], minlength=N).astype(np.float32)
            dinv[g] = (1.0 / (deg + 1.0)).astype(bf)
        in_maps.append(
            {
                "textp": text_p[k * GPC : (k + 1) * GPC],
                "weightb": w_bf,
                "biascol": bias_col,
                "dinvrep": dinv,
                "adj": adj,
            }
        )

    _cache["in_maps"] = in_maps

    from concourse.bass_utils import run_bass_kernel_spmd

    res = run_bass_kernel_spmd(nc, in_maps, list(range(NCORES)))
    out = np.empty((B, N, F), dtype=np.float32)
    for k in range(NCORES):
        for g in range(GPC):
            out[k * GPC + g] = res.results[k]["outT"][g].astype(np.float32).T
    return out


# revision 10
# speedup vs baseline: 6.8405x; 1.0385x over previous
"""DglGraphConvolution Trainium2 kernel — dense-adjacency matmul.

Math:  out[b] = (A_bᵀ @ (text_b @ W)) * dinv_b + bias,  dinv = 1/(deg+1)
Computed as  outᵀ = hidᵀ @ A  with  hid = text @ W,  so every matmul
uses natural layouts (no on-chip transposes):

  1. Host ships, per graph: dense adjacency counts A[src, dst] as
     fp8_e4m3 (exact small ints; 16.8 MB vs 128 MB of one-hot tiles),
     textᵀ [fin, node] bf16 (host pre-transpose), and 1/(deg+1)
     replicated across partitions (bf16).
  2. Prologue (PSUM free): hid[node, f] = Σ textᵀ-chunkᵀ @ W for both
     graphs — lhsT = textᵀ slice, rhs = stationary W; 4 windows share
     one PSUM bank, DVE evacuates to bf16 SBUF.
  3. Main: outᵀ[f, dst] = Σ_ws hid[ws]ᵀ @ A[ws] — lhsT = hid slice
     [ns, f] (natural), rhs = fp8 adjacency slab, free dim 512; all 8
     PSUM banks accumulate one graph's full [128, 4096] result.
  4. Tail per 512-chunk: DVE multiplies PSUM by dinv, ACT adds
     per-partition bias (emits bf16), DMA out [F, N].
  5. Host transposes each graph's [F, N] result back to [N, F].

DMA traffic is split across both HWDGE queues (SP + Activation).
Data-parallel over B: 2 graphs per core, 8 cores.
"""

import numpy as np

B, N, E, F = 16, 4096, 131072, 128
NCORES = 8
GPC = B // NCORES  # graphs per core
W = 128  # src window (partition) size
NW = N // W  # 32
NB = N // 512  # 8 psum banks / 512-wide output chunks

_cache = {}


def _build_program():
    from contextlib import ExitStack

    import concourse.bacc as bacc
    import concourse.tile as tile
    from concourse import mybir
    from concourse._compat import get_trn_type

    f32 = mybir.dt.float32
    bf16 = mybir.dt.bfloat16
    f8 = mybir.dt.float8e4

    nc = bacc.Bacc(get_trn_type() or "TRN2", target_bir_lowering=False, debug=False)

    textT_d = nc.dram_tensor("textT", [GPC, F, N], bf16, kind="ExternalInput")
    w_d = nc.dram_tensor("weightb", [F, F], bf16, kind="ExternalInput")
    bias_d = nc.dram_tensor("biascol", [F, 1], f32, kind="ExternalInput")
    dinv_d = nc.dram_tensor("dinvrep", [GPC, 128, N], bf16, kind="ExternalInput")
    adj_d = nc.dram_tensor("adj", [GPC, NW, W, N], f8, kind="ExternalInput")
    out_d = nc.dram_tensor("outT", [GPC, F, N], bf16, kind="ExternalOutput")

    with tile.TileContext(nc) as tc, ExitStack() as ctx:
        const = ctx.enter_context(tc.tile_pool(name="const", bufs=1))
        tpool = ctx.enter_context(tc.tile_pool(name="tpool", bufs=2))
        hpool = ctx.enter_context(tc.tile_pool(name="hpool", bufs=2))
        dpool = ctx.enter_context(tc.tile_pool(name="dpool", bufs=2))
        mpool = ctx.enter_context(tc.tile_pool(name="mpool", bufs=10))
        opool = ctx.enter_context(tc.tile_pool(name="opool", bufs=6))

        w_sb = const.tile([128, F], bf16)
        nc.sync.dma_start(w_sb[:], w_d[:, :])
        bias_sb = const.tile([128, 1], f32)
        nc.sync.dma_start(bias_sb[:], bias_d[:, :])

        # textᵀ for both graphs, split halves across both DMA queues
        textT_sb = []
        for g in range(GPC):
            t_sb = tpool.tile([128, N], bf16, tag="textT", name=f"textT{g}")
            nc.sync.dma_start(t_sb[:, 0 : N // 2], textT_d[g, :, 0 : N // 2])
            nc.scalar.dma_start(t_sb[:, N // 2 : N], textT_d[g, :, N // 2 : N])
            textT_sb.append(t_sb)

        dinv_sb = []
        for g in range(GPC):
            d_sb = dpool.tile([128, N], bf16, tag="dinv", name=f"dinv{g}")
            nc.scalar.dma_start(d_sb[:], dinv_d[g])
            dinv_sb.append(d_sb)

        # prologue: hid = text @ W for both graphs while PSUM is free
        hid_sb = []
        with tc.tile_pool(name="hpsum", bufs=2, space="PSUM") as hpsum:
            for g in range(GPC):
                h_sb = hpool.tile([128, N], bf16, tag="hid", name=f"hid{g}")
                for c in range(NB):
                    hq = hpsum.tile([128, 512], f32, tag="hq", name=f"hq{g}_{c}")
                    for j in range(4):
                        ws = 4 * c + j
                        nc.tensor.matmul(
                            out=hq[:, 128 * j : 128 * (j + 1)],
                            lhsT=textT_sb[g][:, 128 * ws : 128 * (ws + 1)],
                            rhs=w_sb[:],
                            start=True,
                            stop=True,
                        )
                    nc.vector.tensor_copy(
                        h_sb[:, 512 * c : 512 * (c + 1)], hq[:]
                    )
                hid_sb.append(h_sb)

        with tc.tile_pool(name="psum", bufs=1, space="PSUM") as psum:
            for g in range(GPC):
                # outᵀ[f, dst] accumulated across all ws into 8 psum banks
                P = [
                    psum.tile([128, 512], f32, tag=f"P{b}", name=f"P{g}_{b}")
                    for b in range(NB)
                ]
                for ws in range(NW):
                    m_sb = mpool.tile([128, N], f8, tag="m", name=f"m{g}_{ws}")
                    eng = nc.sync if ws % 2 == 0 else nc.scalar
                    eng.dma_start(m_sb[:], adj_d[g, ws])
                    for b in range(NB):
                        nc.tensor.matmul(
                            out=P[b][:],
                            lhsT=hid_sb[g][:, 128 * ws : 128 * (ws + 1)],
                            rhs=m_sb[:, 512 * b : 512 * (b + 1)],
                            start=(ws == 0),
                            stop=(ws == NW - 1),
                        )

                for b in range(NB):
                    o1 = opool.tile([128, 512], f32, tag="o1", name=f"o1_{g}_{b}")
                    nc.vector.tensor_tensor(
                        out=o1[:],
                        in0=P[b][:],
                        in1=dinv_sb[g][:, 512 * b : 512 * (b + 1)],
                        op=mybir.AluOpType.mult,
                    )
                    o3 = opool.tile([128, 512], bf16, tag="o3", name=f"o3_{g}_{b}")
                    nc.scalar.activation(
                        o3[:],
                        o1[:],
                        mybir.ActivationFunctionType.Identity,
                        bias=bias_sb[:],
                    )
                    nc.sync.dma_start(out_d[g, :, 512 * b : 512 * (b + 1)], o3[:])

    nc.compile()
    return nc


def kernel(text, weight, bias, edge_src, edge_dst):
    import ml_dtypes

    text = np.asarray(text, dtype=np.float32)
    weight = np.asarray(weight, dtype=np.float32)
    bias = np.asarray(bias, dtype=np.float32)
    edge_src = np.asarray(edge_src, dtype=np.int64)
    edge_dst = np.asarray(edge_dst, dtype=np.int64)

    if "nc" not in _cache:
        _cache["nc"] = _build_program()
    nc = _cache["nc"]

    fp8 = ml_dtypes.float8_e4m3
    bf = ml_dtypes.bfloat16
    lut = np.arange(64, dtype=np.float32).astype(fp8)  # exact ints through 16+

    # [B, N, F] -> [B, F, N]
    text_T = np.ascontiguousarray(text.astype(bf).transpose(0, 2, 1))
    w_bf = weight.astype(bf)
    bias_col = bias.astype(np.float32).reshape(F, 1)

    in_maps = []
    for k in range(NCORES):
        adj = np.empty((GPC, NW, W, N), dtype=fp8)
        dinv = np.empty((GPC, 128, N), dtype=bf)
        for g in range(GPC):
            b = k * GPC + g
            cnt = np.bincount(
                edge_src[b] * N + edge_dst[b], minlength=N * N
            )
            assert cnt.max() < 16, f"edge multiplicity {cnt.max()} too large"
            adj[g] = lut[cnt].reshape(NW, W, N)
            deg = np.bincount(edge_dst[b], minlength=N).astype(np.float32)
            dinv[g] = (1.0 / (deg + 1.0)).astype(bf)
        in_maps.append(
            {
                "textT": text_T[k * GPC : (k + 1) * GPC],
                "weightb": w_bf,
                "biascol": bias_col,
                "dinvrep": dinv,
                "adj": adj,
            }
        )

    _cache["in_maps"] = in_maps

    from concourse.bass_utils import run_bass_kernel_spmd

    res = run_bass_kernel_spmd(nc, in_maps, list(range(NCORES)))
    out = np.empty((B, N, F), dtype=np.float32)
    for k in range(NCORES):
        for g in range(GPC):
            out[k * GPC + g] = res.results[k]["outT"][g].astype(np.float32).T
    return out
